# revision 35
# baseline (speedup 1.0000x reference)
"""Trainium2 Bass kernel for nn_CNN_RNN_88347477278730.

Pipeline (data-parallel over batch, 8 rows per core on 8 cores):
  kernel1 (fp32): input projection hoisted, then the select-policy GRUCell
      recurrence (t_len1 = maxlen-1 steps). Per step the gate PSUM is
      preloaded with gi_rz / bhh_n on the scalar engine and the h@Whh
      matmuls accumulate on top (start=False); the elementwise chain uses
      1-z = sigmoid(-x) so the critical path is 6 ops. All h_t are stored
      and the Gumbel decision bits are computed in bulk at the end.
  host: compaction (gather kept tokens to the front), new_lens, Ldyn.
  kernel2 (fp16 matmuls, t_len2 = Ldyn rounded up): proj of compacted
      embeddings, 2-layer GRU recurrences (h stored fp16), valid-masking,
      Kim-CNN convs as shifted matmuls, masked max-pool, final linear.

Matmul layouts are weights-stationary: lhsT = weight tiles [K=128, M=128],
moving operand = transposed activations [K, batch], so gate tensors land
partition-major where the elementwise engines are fast.
"""

import os
import subprocess
import sys
import tempfile

import numpy as np

# ---------------------------------------------------------------- constants
B, T, E, H, NF = 64, 512, 768, 256, 100
NCORES = 8
BPC = B // NCORES  # batch rows per core
KE = E // 128      # 6 K-tiles over the embedding dim
KH = H // 128      # 2 K-tiles over the hidden dim
GC = (3 * H) // 128  # 6 gate chunks (r: 0-1, z: 2-3, n: 4-5)
NEG = -1.0e30

FS = (3, 4, 5)


# ------------------------------------------------------------- tile patch
def _apply_tile_patch():
    """This walrus build rejects >2 sem waits on one SP control instruction;
    split the TileContext tail drain into several drains of <=2 waits."""
    import concourse.tile as tile
    from concourse.vector_clock import ScopedClock, VectorClock

    if getattr(tile.TileContext, "_drain_split_patched", False):
        return

    def _patched(self, tick_clock, wait_clock):
        gc = tick_clock.global_clock
        n = len(gc)
        for start in range(0, n, 1):
            vec = [0] * n
            any_set = False
            for p in range(start, min(start + 1, n)):
                vec[p] = gc[p]
                any_set = any_set or vec[p] > 0
            if not any_set:
                continue
            d = self.nc.sync.drain()
            wait_clock.add_sem_waits(d.ins, ScopedClock({None: VectorClock(vec)}))
        self.nc.all_engine_barrier()
        assert self.sems is not None
        popped = self.nc._tile_sem_poison_stack.pop()
        assert popped is self._sem_poison
        self.nc.clear_and_free_semaphores(list(self.sems.allocated().values()))
        self.nc.all_engine_barrier()

    tile.TileContext._drain_and_barrier = _patched
    tile.TileContext._drain_split_patched = True


# ------------------------------------------------------------- gumbel (CPU)
def _gumbel_cpu():
    """jax.random.gumbel(key(42), (T-1, B, 2), f32) — computed in a CPU-jax
    subprocess so the accelerator backend is never involved (it must be
    bit-identical to the reference's CPU computation)."""
    path = os.path.join(tempfile.mkdtemp(), "gumbel.npy")
    code = (
        "import numpy as np, jax, jax.numpy as jnp\n"
        f"g = jax.random.gumbel(jax.random.key(42), ({T - 1}, {B}, 2), jnp.float32)\n"
        f"np.save({path!r}, np.asarray(g))\n"
    )
    env = dict(os.environ)
    env["TRN_TERMINAL_POOL_IPS"] = ""
    env["JAX_PLATFORMS"] = "cpu"
    extra = [p for p in sys.path if p and os.path.isdir(p)]
    env["PYTHONPATH"] = os.pathsep.join(extra)
    subprocess.run([sys.executable, "-c", code], env=env, check=True, capture_output=True)
    return np.load(path)


# ------------------------------------------------------------- host packing
def _pack_T(a2d):
    """[rows(=128*k), cols] -> [128, k, cols] weight-tile layout."""
    rows, cols = a2d.shape
    k = rows // 128
    return np.ascontiguousarray(a2d.reshape(k, 128, cols).transpose(1, 0, 2)).astype(np.float32)


def _pack_bias(b1d):
    """[128*k] -> [128, k]"""
    k = b1d.shape[0] // 128
    return np.ascontiguousarray(b1d.reshape(k, 128).T).astype(np.float32)


def _pack_embT(emb_rows, t_len):
    """[bpc, t_len, E] -> [KE, 128, bpc*t_len] (e-major tiles, free (b, t))."""
    bpc = emb_rows.shape[0]
    x = emb_rows.transpose(2, 0, 1).reshape(KE, 128, bpc * t_len)
    return np.ascontiguousarray(x)


def _pack_gru_weights(Wih, Whh, bih, bhh):
    """Returns (wihT, whhT, bias_proj, bhhn_rep).

    bias_proj folds bih+bhh for the r,z chunks (added once at projection
    time); n chunks get bih only, with bhh_n preloaded into PSUM per step
    (it must be added to h@Whh_n *before* the r* multiply)."""
    wihT = _pack_T(np.ascontiguousarray(Wih.T))  # [128, KE or KH, 3H]
    whhT = _pack_T(np.ascontiguousarray(Whh.T))  # [128, KH, 3H]
    bias = np.empty(3 * H, np.float32)
    bias[: 2 * H] = bih[: 2 * H] + bhh[: 2 * H]
    bias[2 * H :] = bih[2 * H :]
    bias_proj = _pack_bias(bias)  # [128, GC]
    bhhn = _pack_bias(bhh[2 * H :])  # [128, KH]
    bhhn_rep = np.ascontiguousarray(
        np.broadcast_to(bhhn[:, :, None], (128, KH, BPC))
    ).astype(np.float32)
    return wihT, whhT, bias_proj, bhhn_rep


# ------------------------------------------------------------- bass builders
def _mk_nc():
    import concourse.bass as bass

    return bass.Bass("TRN2", target_bir_lowering=False, debug=False, num_devices=1)


def _split_excess_waits(nc, max_waits=1):
    """This walrus build can only encode ~2 sem waits per instruction
    (setupSyncWait 'Too many sync wait commands'). Hoist excess waits onto
    same-engine NoOps inserted just before the over-subscribed instruction;
    engine queues execute in order, so the wait semantics are identical."""
    from concourse import mybir

    nid = [0]
    for f in nc.m.functions:
        for bb in f.blocks:
            out = []
            changed = False
            for inst in bb.instructions:
                si = inst.sync_info
                lim = max_waits
                if si is not None and si.on_wait and len(si.on_wait) > lim:
                    waits = list(si.on_wait)
                    extra, keep = waits[:-lim], waits[-lim:]
                    for j in range(0, len(extra), max_waits):
                        nop = mybir.InstNoOp(
                            name=f"I-waitnop-{nid[0]}", ins=[], outs=[])
                        nid[0] += 1
                        nop.engine = inst.engine
                        nop.sync_info = mybir.SyncInfo(
                            on_wait=extra[j : j + max_waits], on_update=[])
                        nc.register_instruction(nop, overwrite=True)
                        out.append(nop)
                    inst.sync_info = mybir.SyncInfo(
                        on_wait=keep, on_update=list(si.on_update or []))
                    changed = True
                out.append(inst)
            if changed:
                bb.instructions = out
    return nc


def _proj_block_closures(nc, mybir, embT_d, wihcT, biasc, gi_rz, gin, lo, hi,
                         t_len, dma_pool, proj_ps, dt_in=None):
    """Closures that emit the input projection for t in [lo, hi).
    Returned as a list so the caller can pace them into the recurrence's
    PE idle gaps (each closure is one DMA / one matmul / one activation)."""
    f32 = mybir.dt.float32
    act = mybir.ActivationFunctionType
    if dt_in is None:
        dt_in = f32
    w = hi - lo
    cls = []
    for b in range(BPC):
        srch = []

        def _dma(srch=srch, b=b, lo=lo, w=w):
            src = dma_pool.tile([128, KE, w], dt_in, tag="projsrc", name="psrc")
            for k in range(KE):
                nc.sync.dma_start(
                    src[:, k, :],
                    embT_d[k, :, b * t_len + lo : b * t_len + lo + w])
            srch.append(src)

        cls.append(_dma)
        for c in range(GC):
            psh = []
            for k in range(KE):
                def _mm(srch=srch, psh=psh, c=c, k=k, w=w):
                    if k == 0:
                        psh.append(proj_ps.tile([128, w], f32, tag="projps", name="pps"))
                    nc.tensor.matmul(
                        psh[0][:], wihcT[:, k, c * 128 : (c + 1) * 128],
                        srch[0][:, k, :], start=(k == 0), stop=(k == KE - 1))
                cls.append(_mm)

            def _act(psh=psh, c=c, b=b, lo=lo, hi=hi):
                dst = gi_rz[:, lo:hi, c, b] if c < 4 else gin[:, lo:hi, c - 4, b]
                nc.scalar.activation(dst, psh[0][:], act.Identity,
                                     bias=biasc[:, c : c + 1])

            cls.append(_act)
    return cls


def _pace(sched, work, start, end):
    """Spread `work` closures evenly over steps [start, end]."""
    span = max(1, end - start)
    n = len(work)
    for j, cl in enumerate(work):
        sched.setdefault(start + (j * span) // n, []).append(cl)


def _emit_proj(nc, mybir, src_dram, wT_sb, bias_sb, gi_rz, gin, kin, t_len,
               dma_pool, ps_pool, dt_in):
    """gi[c*128+p, t, b] = sum_e W[e, c*128+p] * src[e, b, t] + bias.

    src_dram: DRAM [kin, 128, BPC*t_len]; wT_sb: [128, kin, 3H];
    gi_rz: [128, t_len, 4, BPC] (chunks 0-3); gin: [128, t_len, 2, BPC]."""
    f32 = mybir.dt.float32
    act = mybir.ActivationFunctionType
    for b in range(BPC):
        src_sb = dma_pool.tile([128, kin, t_len], dt_in, tag="projsrc")
        for k in range(kin):
            nc.sync.dma_start(
                src_sb[:, k, :], src_dram[k, :, b * t_len : (b + 1) * t_len]
            )
        for c in range(GC):
            ps = ps_pool.tile([128, t_len], f32, tag="projps")
            for k in range(kin):
                nc.tensor.matmul(
                    ps[:],
                    wT_sb[:, k, c * 128 : (c + 1) * 128],
                    src_sb[:, k, :],
                    start=(k == 0),
                    stop=(k == kin - 1),
                )
            dst = gi_rz[:, :, c, b] if c < 4 else gin[:, :, c - 4, b]
            nc.scalar.activation(dst, ps[:], act.Identity, bias=bias_sb[:, c : c + 1])


def _emit_gru_step(nc, mybir, t, h_prev, gi_rz, gin, whh_sb, bhhn_sb,
                   step_ps, sb_pool, h_out, fast_tail=False):
    """One GRU step. h_prev: AP [128, KH, BPC] of h_{t-1} (None for h=0).
    Writes h_t to h_out ([128, KH, BPC] view, any dtype).

    step_ps = (rz_pool, n_pool): the rz and n gate accumulations live in
    separate PSUM tiles so the rz add + sigmoid can start while the PE is
    still on the n-chunk matmuls. PSUM is written by the PE only — non-PE
    PSUM preloads raced with PE accumulation (nondeterministic results).

    fast_tail=False: baseline-exact numerics — gi added on DVE then sigmoid
    in place, and h' = n + z*(h-n). Required for the decision-exact select
    kernel. hn = psum + bhh_n is one DVE add (bit-identical to the Act
    bias-add it replaces, both are fp32 IEEE adds).
    fast_tail=True: 1-z = sigmoid(-zraw), h' = (1-z)*n + z*h with z*h off
    the critical path (1e-7-class deviation from the reference op order)."""
    f32 = mybir.dt.float32
    act = mybir.ActivationFunctionType
    alu = mybir.AluOpType

    rz_pool, n_pool = step_ps
    rz_ps = rz_pool.tile([128, 4, BPC], f32, tag="rzps")
    n_ps = n_pool.tile([128, 2, BPC], f32, tag="nps")
    if h_prev is not None:
        # chunk-major only: each column's accumulation group must be
        # contiguous — interleaving open groups in one PSUM bank corrupts
        # the accumulation (start resets bank-level group state)
        order = [(c, k) for c in range(GC) for k in range(KH)]
        for c, k in order:
            ps = rz_ps[:, c, :] if c < 4 else n_ps[:, c - 4, :]
            nc.tensor.matmul(
                ps,
                whh_sb[:, k, c * 128 : (c + 1) * 128],
                h_prev[:, k, :],
                start=(k == 0),
                stop=(k == KH - 1),
            )

    rz = sb_pool.tile([128, 4, BPC], f32, tag="rz")
    hn_src = bhhn_sb[:, :, :]
    if fast_tail:
        omz = sb_pool.tile([128, 2, BPC], f32, tag="omz")
        if h_prev is not None:
            raw = sb_pool.tile([128, 4, BPC], f32, tag="raw")
            nc.vector.tensor_tensor(raw[:], rz_ps[:], gi_rz[:, t, :, :], alu.add)
            nc.scalar.activation(rz[:], raw[:], act.Sigmoid)
            hn = sb_pool.tile([128, 2, BPC], f32, tag="hn")
            nc.vector.tensor_tensor(hn[:], n_ps[:], bhhn_sb, alu.add)
            hn_src = hn[:]
            nc.scalar.activation(omz[:], raw[:, 2:4, :], act.Sigmoid, scale=-1.0)
        else:
            nc.scalar.activation(rz[:], gi_rz[:, t, :, :], act.Sigmoid)
            nc.scalar.activation(omz[:], gi_rz[:, t, 2:4, :], act.Sigmoid, scale=-1.0)
        t1 = sb_pool.tile([128, 2, BPC], f32, tag="t1")
        nc.vector.tensor_tensor(t1[:], rz[:, 0:2, :], hn_src, alu.mult)
        t2 = sb_pool.tile([128, 2, BPC], f32, tag="t2")
        nc.vector.tensor_tensor(t2[:], t1[:], gin[:, t, :, :], alu.add)
        if h_prev is not None:
            # on gpsimd: off the serial path, keeps DVE free for the chain
            zh = sb_pool.tile([128, 2, BPC], f32, tag="zh")
            nc.gpsimd.tensor_tensor(zh[:], rz[:, 2:4, :], h_prev, alu.mult)
        nn_ = sb_pool.tile([128, 2, BPC], f32, tag="nn")
        nc.scalar.activation(nn_[:], t2[:], act.Tanh)
        if h_prev is None:
            nc.vector.tensor_tensor(h_out, omz[:], nn_[:], alu.mult)
        else:
            f1 = sb_pool.tile([128, 2, BPC], f32, tag="f1")
            nc.gpsimd.tensor_tensor(f1[:], omz[:], nn_[:], alu.mult)
            nc.vector.tensor_tensor(h_out, f1[:], zh[:], alu.add)
        return

    # exact path (select kernel), k-split: the n-gate chain runs per
    # 128-dim half so the k0 half of h' lands ~1us before the k1 half and
    # the next step's k0 matmul round starts under the k1 tail. Per-element
    # arithmetic is identical to the unsplit form.
    if h_prev is not None:
        nc.vector.tensor_tensor(rz[:], rz_ps[:], gi_rz[:, t, :, :], alu.add)
        nc.scalar.activation(rz[:], rz[:], act.Sigmoid)
    else:
        nc.scalar.activation(rz[:], gi_rz[:, t, :, :], act.Sigmoid)
    t1 = sb_pool.tile([128, 2, BPC], f32, tag="t1")
    t2 = sb_pool.tile([128, 2, BPC], f32, tag="t2")
    nn_ = sb_pool.tile([128, 2, BPC], f32, tag="nn")
    hn = sb_pool.tile([128, 2, BPC], f32, tag="hn")
    for k in range(KH):
        if h_prev is not None:
            nc.vector.tensor_tensor(hn[:, k, :], n_ps[:, k, :],
                                    bhhn_sb[:, k, :], alu.add)
            hsrc = hn[:, k, :]
        else:
            hsrc = bhhn_sb[:, k, :]
        nc.vector.tensor_tensor(t1[:, k, :], rz[:, k, :], hsrc, alu.mult)
        nc.vector.tensor_tensor(t2[:, k, :], t1[:, k, :], gin[:, t, k, :], alu.add)
        nc.scalar.activation(nn_[:, k, :], t2[:, k, :], act.Tanh)
    d = sb_pool.tile([128, 2, BPC], f32, tag="dd")
    for k in range(KH):
        if h_prev is None:
            nc.vector.tensor_scalar(d[:, k, :], nn_[:, k, :], -1.0, None, alu.mult)
        else:
            nc.vector.tensor_tensor(d[:, k, :], h_prev[:, k, :], nn_[:, k, :],
                                    alu.subtract)
        nc.vector.tensor_tensor(d[:, k, :], rz[:, 2 + k, :], d[:, k, :], alu.mult)
        nc.vector.tensor_tensor(h_out[:, k, :], nn_[:, k, :], d[:, k, :], alu.add)


def build_kernel1(t_len):
    """Select-policy kernel: proj + recurrence + bulk decisions. fp32."""
    import concourse.tile as tile
    from concourse import mybir

    _apply_tile_patch()
    nc = _mk_nc()
    f32 = mybir.dt.float32
    alu = mybir.AluOpType

    embT_d = nc.dram_tensor("embT", [KE, 128, BPC * t_len], f32, kind="ExternalInput").ap()
    wihcT_d = nc.dram_tensor("wihcT", [128, KE, 3 * H], f32, kind="ExternalInput").ap()
    whhcT_d = nc.dram_tensor("whhcT", [128, KH, 3 * H], f32, kind="ExternalInput").ap()
    wdiffT_d = nc.dram_tensor("wdiffT", [128, KH, 1], f32, kind="ExternalInput").ap()
    biasc_d = nc.dram_tensor("biasc", [128, GC], f32, kind="ExternalInput").ap()
    bhhnc_d = nc.dram_tensor("bhhnc", [128, KH, BPC], f32, kind="ExternalInput").ap()
    ncdiff_d = nc.dram_tensor("ncdiff", [1, BPC * t_len], f32, kind="ExternalInput").ap()
    ks_d = nc.dram_tensor("ks", [1, BPC * t_len], f32, kind="ExternalOutput").ap()

    with tile.TileContext(nc) as tc:
        from contextlib import ExitStack

        with ExitStack() as ctx:
            wpool = ctx.enter_context(tc.tile_pool(name="weights", bufs=1))
            gipool = ctx.enter_context(tc.tile_pool(name="gi", bufs=1))
            dma_pool = ctx.enter_context(tc.tile_pool(name="dma", bufs=2))
            proj_ps = ctx.enter_context(tc.tile_pool(name="projps", bufs=2, space="PSUM"))
            rz_pool = ctx.enter_context(tc.tile_pool(name="rzps", bufs=2, space="PSUM"))
            n_pool = ctx.enter_context(tc.tile_pool(name="nps", bufs=2, space="PSUM"))
            step_ps = (rz_pool, n_pool)
            lg_ps = ctx.enter_context(tc.tile_pool(name="lgps", bufs=2, space="PSUM"))
            sb_pool = ctx.enter_context(tc.tile_pool(name="gates", bufs=2))
            misc = ctx.enter_context(tc.tile_pool(name="misc", bufs=1))

            def _load(pool, dram, shape, tag, dt=f32):
                t_ = pool.tile(shape, dt, tag=tag)
                nc.sync.dma_start(t_[:], dram[:])
                return t_

            wihcT = _load(wpool, wihcT_d, [128, KE, 3 * H], "wihcT")
            whhcT = _load(wpool, whhcT_d, [128, KH, 3 * H], "whhcT")
            wdiffT = _load(wpool, wdiffT_d, [128, KH, 1], "wdiffT")
            biasc = _load(wpool, biasc_d, [128, GC], "biasc")
            bhhnc = _load(wpool, bhhnc_d, [128, KH, BPC], "bhhnc")
            ncdiff = _load(misc, ncdiff_d, [1, BPC * t_len], "ncdiff")

            gi_rz = gipool.tile([128, t_len, 4, BPC], f32, tag="girz")
            gin = gipool.tile([128, t_len, 2, BPC], f32, tag="gin")
            h_all = gipool.tile([128, KH, BPC, t_len], f32, tag="hall")

            # t-blocked proj: block 0 upfront, later blocks paced into the
            # recurrence's PE idle gaps (block i must land before step bnds[i])
            bnds = list(range(0, t_len, 128))
            if bnds[-1] != t_len:
                bnds.append(t_len)
            blocks = [(bnds[i], bnds[i + 1]) for i in range(len(bnds) - 1)]
            for cl in _proj_block_closures(nc, mybir, embT_d, wihcT, biasc,
                                           gi_rz, gin, blocks[0][0], blocks[0][1],
                                           t_len, dma_pool, proj_ps):
                cl()
            sched = {}
            for i in range(1, len(blocks)):
                lo, hi = blocks[i]
                work = _proj_block_closures(nc, mybir, embT_d, wihcT, biasc,
                                            gi_rz, gin, lo, hi, t_len,
                                            dma_pool, proj_ps)
                _pace(sched, work, blocks[i - 1][0] + 2, lo - 4)

            h_prev = None
            for t in range(t_len):
                h_out = h_all[:, :, :, t]
                _emit_gru_step(nc, mybir, t, h_prev, gi_rz, gin, whhcT, bhhnc,
                               step_ps, sb_pool, h_out)
                for cl in sched.pop(t, []):
                    cl()
                h_prev = h_out
            for rest in sorted(sched):
                for cl in sched.pop(rest):
                    cl()

            # bulk decision readout: ks[b, t] = (h_t . wdiff > ncdiff[b, t])
            ks_sb = misc.tile([1, BPC * t_len], f32, tag="kssb")
            for b in range(BPC):
                lg = lg_ps.tile([1, t_len], f32, tag="lg")
                for k in range(KH):
                    nc.tensor.matmul(
                        lg[:],
                        wdiffT[:, k, :],
                        h_all[:, k, b, :],
                        start=(k == 0),
                        stop=(k == KH - 1),
                    )
                nc.vector.tensor_tensor(
                    ks_sb[0:1, b * t_len : (b + 1) * t_len], lg[:],
                    ncdiff[0:1, b * t_len : (b + 1) * t_len], alu.is_gt
                )
            nc.sync.dma_start(ks_d[:], ks_sb[:])

    return _split_excess_waits(nc)


def build_kernel2(t_len):
    """GRU0/GRU1 + convs + pooling + final linear. fp16 matmuls."""
    import concourse.tile as tile
    from concourse import mybir

    _apply_tile_patch()
    nc = _mk_nc()
    f32 = mybir.dt.float32
    f16 = mybir.dt.float16
    act = mybir.ActivationFunctionType
    alu = mybir.AluOpType

    nembT_d = nc.dram_tensor("nembT", [KE, 128, BPC * t_len], f16, kind="ExternalInput").ap()
    wih0T_d = nc.dram_tensor("wih0T", [128, KE, 3 * H], f16, kind="ExternalInput").ap()
    whh0T_d = nc.dram_tensor("whh0T", [128, KH, 3 * H], f16, kind="ExternalInput").ap()
    bias0_d = nc.dram_tensor("bias0", [128, GC], f32, kind="ExternalInput").ap()
    bhhn0_d = nc.dram_tensor("bhhn0", [128, KH, BPC], f32, kind="ExternalInput").ap()
    wih1T_d = nc.dram_tensor("wih1T", [128, KH, 3 * H], f16, kind="ExternalInput").ap()
    whh1T_d = nc.dram_tensor("whh1T", [128, KH, 3 * H], f16, kind="ExternalInput").ap()
    bias1_d = nc.dram_tensor("bias1", [128, GC], f32, kind="ExternalInput").ap()
    bhhn1_d = nc.dram_tensor("bhhn1", [128, KH, BPC], f32, kind="ExternalInput").ap()
    vt_d = nc.dram_tensor("vt", [1, BPC * t_len], f32, kind="ExternalInput").ap()
    cw_d = nc.dram_tensor("cw", [128, 12, KH, NF], f16, kind="ExternalInput").ap()
    cb_d = nc.dram_tensor("cb", [NF, 3], f32, kind="ExternalInput").ap()
    tmask_d = nc.dram_tensor("tmask", [NF, 3, t_len], f32, kind="ExternalInput").ap()
    woutT_d = nc.dram_tensor("woutT", [NF, 3], f32, kind="ExternalInput").ap()
    bout_d = nc.dram_tensor("bout", [1, 1], f32, kind="ExternalInput").ap()
    out_d = nc.dram_tensor("out", [1, BPC], f32, kind="ExternalOutput").ap()

    with tile.TileContext(nc) as tc:
        from contextlib import ExitStack

        with ExitStack() as ctx:
            wpool = ctx.enter_context(tc.tile_pool(name="weights", bufs=1))
            gipool = ctx.enter_context(tc.tile_pool(name="gi", bufs=1))
            opool = ctx.enter_context(tc.tile_pool(name="obuf", bufs=1))
            dma_pool = ctx.enter_context(tc.tile_pool(name="dma", bufs=2))
            proj_ps = ctx.enter_context(tc.tile_pool(name="projps", bufs=2, space="PSUM"))
            rz_pool = ctx.enter_context(tc.tile_pool(name="rzps", bufs=2, space="PSUM"))
            n_pool = ctx.enter_context(tc.tile_pool(name="nps", bufs=2, space="PSUM"))
            step_ps = (rz_pool, n_pool)
            fin_ps = ctx.enter_context(tc.tile_pool(name="finps", bufs=1, space="PSUM"))
            sb_pool = ctx.enter_context(tc.tile_pool(name="gates", bufs=3))
            misc = ctx.enter_context(tc.tile_pool(name="misc", bufs=1))

            def _load(pool, dram, shape, tag, dt=f32):
                t_ = pool.tile(shape, dt, tag=tag)
                nc.sync.dma_start(t_[:], dram[:])
                return t_

            wih0T = _load(wpool, wih0T_d, [128, KE, 3 * H], "bigw", f16)
            whh0T = _load(wpool, whh0T_d, [128, KH, 3 * H], "whh0", f16)
            bias0 = _load(wpool, bias0_d, [128, GC], "bias0")
            bhhn0 = _load(wpool, bhhn0_d, [128, KH, BPC], "bhhn0")
            wih1T = _load(wpool, wih1T_d, [128, KH, 3 * H], "wih1", f16)
            whh1T = _load(wpool, whh1T_d, [128, KH, 3 * H], "whh1", f16)
            bias1 = _load(wpool, bias1_d, [128, GC], "bias1")
            bhhn1 = _load(wpool, bhhn1_d, [128, KH, BPC], "bhhn1")
            cb = _load(misc, cb_d, [NF, 3], "cb")
            tmask = _load(misc, tmask_d, [NF, 3, t_len], "tmask")
            woutT = _load(misc, woutT_d, [NF, 3], "woutT")
            bout = _load(misc, bout_d, [1, 1], "bout")
            vt = _load(misc, vt_d, [1, BPC * t_len], "vt")

            # ---- layer 0 + pipelined layer 1 (lagged by LAG steps) ----
            gi_rz0 = gipool.tile([128, t_len, 4, BPC], f32, tag="girz0")
            gin0 = gipool.tile([128, t_len, 2, BPC], f32, tag="gin0")
            _emit_proj(nc, mybir, nembT_d, wih0T, bias0, gi_rz0, gin0, KE, t_len,
                       dma_pool, proj_ps, f16)
            o1 = opool.tile([128, KH, BPC, t_len], f16, tag="o1")
            o2 = opool.tile([128, KH, BPC, t_len], f16, tag="o2")
            gi_rz1 = gipool.tile([128, t_len, 4, BPC], f32, tag="girz1")
            gin1 = gipool.tile([128, t_len, 2, BPC], f32, tag="gin1")

            PB, LAG = 64, 96

            def proj1_block_closures(lo, hi):
                cls = []
                for b in range(BPC):
                    for c in range(GC):
                        psh = []
                        for k in range(KH):
                            def _mm(psh=psh, c=c, b=b, k=k, lo=lo, hi=hi):
                                if k == 0:
                                    psh.append(proj_ps.tile(
                                        [128, hi - lo], f32, tag="projps",
                                        name="pps1"))
                                nc.tensor.matmul(
                                    psh[0][:], wih1T[:, k, c * 128 : (c + 1) * 128],
                                    o1[:, k, b, lo:hi],
                                    start=(k == 0), stop=(k == KH - 1))
                            cls.append(_mm)

                        def _act(psh=psh, c=c, b=b, lo=lo, hi=hi):
                            dst = (gi_rz1[:, lo:hi, c, b] if c < 4
                                   else gin1[:, lo:hi, c - 4, b])
                            nc.scalar.activation(dst, psh[0][:], act.Identity,
                                                 bias=bias1[:, c : c + 1])
                        cls.append(_act)
                return cls

            sched = {}
            lo = 0
            while lo < t_len:
                hi = min(lo + PB, t_len)
                # o1[:, lo:hi] complete after layer-0 step hi-1; gi1[lo:hi]
                # needed by layer-1 step lo, which runs at merged step lo+LAG
                _pace(sched, proj1_block_closures(lo, hi), hi, lo + LAG - 2)
                lo = hi

            h0_prev = None
            h1_prev = None
            for tt in range(t_len + LAG):
                if tt < t_len:
                    h_out = o1[:, :, :, tt]
                    _emit_gru_step(nc, mybir, tt, h0_prev, gi_rz0, gin0, whh0T,
                                   bhhn0, step_ps, sb_pool, h_out, fast_tail=True)
                    h0_prev = h_out
                for cl in sched.pop(tt, []):
                    cl()
                if tt >= LAG:
                    t1_ = tt - LAG
                    h_out = o2[:, :, :, t1_]
                    _emit_gru_step(nc, mybir, t1_, h1_prev, gi_rz1, gin1, whh1T,
                                   bhhn1, step_ps, sb_pool, h_out, fast_tail=True)
                    h1_prev = h_out

            # ---- zero o2 past new_lens: o2 *= vt ----
            ones_sb = misc.tile([1, 128], f32, tag="ones")
            nc.vector.memset(ones_sb[:], 1.0)
            for b in range(BPC):
                vtb = proj_ps.tile([128, t_len], f32, tag="projps")
                nc.tensor.matmul(
                    vtb[:], ones_sb[:], vt[0:1, b * t_len : (b + 1) * t_len],
                    start=True, stop=True,
                )
                for k in range(KH):
                    nc.vector.tensor_tensor(
                        o2[:, k, b, :], o2[:, k, b, :], vtb[:], alu.mult
                    )

            # ---- convs + relu + tmask + max-pool ----
            cw = _load(wpool, cw_d, [128, 12, KH, NF], "bigw", f16)
            pooled = misc.tile([NF, 3, BPC], f32, tag="pooled")
            for b in range(BPC):
                for fi, fs in enumerate(FS):
                    nw = t_len - fs + 1
                    ps = proj_ps.tile([NF, t_len], f32, tag="projps")
                    m0 = sum(FS[:fi])
                    first = True
                    for dt_ in range(fs):
                        for k in range(KH):
                            nc.tensor.matmul(
                                ps[:, :nw],
                                cw[:, m0 + dt_, k, :],
                                o2[:, k, b, dt_ : dt_ + nw],
                                start=first,
                                stop=(dt_ == fs - 1 and k == KH - 1),
                            )
                            first = False
                    crelu = sb_pool.tile([NF, t_len], f32, tag="crelu")
                    nc.scalar.activation(
                        crelu[:, :nw], ps[:, :nw], act.Relu, bias=cb[:, fi : fi + 1]
                    )
                    nc.vector.tensor_tensor(
                        crelu[:, :nw], crelu[:, :nw], tmask[:, fi, :nw], alu.add
                    )
                    nc.vector.tensor_reduce(
                        pooled[:, fi, b : b + 1], crelu[:, :nw],
                        mybir.AxisListType.X, alu.max
                    )

            # ---- final linear ----
            fps = fin_ps.tile([1, BPC], f32)
            for fi in range(3):
                nc.tensor.matmul(
                    fps[:],
                    woutT[:, fi : fi + 1],
                    pooled[:, fi, :],
                    start=(fi == 0),
                    stop=(fi == 2),
                )
            out_sb = misc.tile([1, BPC], f32, tag="outsb")
            nc.scalar.activation(out_sb[:], fps[:], act.Identity, bias=bout[0:1, 0:1])
            nc.sync.dma_start(out_d[:], out_sb[:])

    return _split_excess_waits(nc)


# ------------------------------------------------------------- host orchestration
def _host_pack_k1(inputs, gumbel):
    emb = np.asarray(inputs["embedded"], np.float32)
    mask = np.asarray(inputs["mask"])
    lens = mask.sum(1)
    maxlen = int(lens.max())
    t_len = maxlen - 1  # h_t needed only for t <= maxlen-2; +1 for t=0 row

    wihcT, whhcT, biasc, bhhnc = _pack_gru_weights(
        inputs["Wih_c"], inputs["Whh_c"], inputs["bih_c"], inputs["bhh_c"])
    wdiff = (inputs["Wsel"][1] - inputs["Wsel"][0]).astype(np.float32)
    wdiffT = np.ascontiguousarray(
        wdiff.reshape(KH, 128).T[:, :, None]).astype(np.float32)
    bdiff = float(inputs["bsel"][1] - inputs["bsel"][0])

    # ncdiff[t, b]: k_t = (h_t . wdiff > ncdiff); t=0 is not a decision
    ncdiff = np.full((t_len, B), 1.0e30, np.float32)
    for t in range(1, t_len):
        ncdiff[t] = -(bdiff + gumbel[t - 1, :, 1] - gumbel[t - 1, :, 0])

    in_maps = []
    for c in range(NCORES):
        rows = slice(c * BPC, (c + 1) * BPC)
        in_maps.append({
            "embT": _pack_embT(emb[rows, :t_len], t_len).astype(np.float32),
            "wihcT": wihcT,
            "whhcT": whhcT,
            "wdiffT": wdiffT,
            "biasc": biasc,
            "bhhnc": bhhnc,
            "ncdiff": np.ascontiguousarray(ncdiff[:, rows].T.reshape(1, BPC * t_len)),
        })
    return in_maps, lens, maxlen, t_len


def _host_compact(inputs, ks_full, lens):
    """ks_full: [B, T] decision bits (col 0 ignored; selected[:,0]=1)."""
    emb = np.asarray(inputs["embedded"], np.float32)
    selected = np.zeros((B, T), np.int64)
    selected[:, 0] = 1
    selected[:, 1:] = ks_full[:, 1:]
    pos = np.arange(T)
    sel_valid = np.where(pos[None, :] < (lens - 1)[:, None], selected, 0)
    new_mask = np.where(pos[None, :] == (lens - 1)[:, None], 1, sel_valid)
    new_lens = new_mask.sum(1)
    Ldyn = max(int(new_lens.max()), 7)

    new_emb = np.zeros((B, T, E), np.float32)
    for b in range(B):
        idx = np.nonzero(new_mask[b])[0]
        new_emb[b, : len(idx)] = emb[b, idx]
    return new_emb, new_lens, Ldyn


def _host_pack_k2(inputs, new_emb, new_lens, Ldyn):
    t_len = min(max(16 * ((Ldyn + 15) // 16), 32), T)

    wih0T, whh0T, bias0, bhhn0 = _pack_gru_weights(
        inputs["Wih0"], inputs["Whh0"], inputs["bih0"], inputs["bhh0"])
    wih1T, whh1T, bias1, bhhn1 = _pack_gru_weights(
        inputs["Wih1"], inputs["Whh1"], inputs["bih1"], inputs["bhh1"])

    cw = np.zeros((128, 12, KH, NF), np.float16)
    cb = np.zeros((NF, 3), np.float32)
    m = 0
    for fi, fs in enumerate(FS):
        w = np.asarray(inputs[f"conv_w{fs}"], np.float32)  # [NF,1,fs,H]
        cb[:, fi] = np.asarray(inputs[f"conv_b{fs}"], np.float32)
        for dt_ in range(fs):
            wt = w[:, 0, dt_, :].T  # [H, NF]
            cw[:, m, :, :] = wt.reshape(KH, 128, NF).transpose(1, 0, 2).astype(np.float16)
            m += 1

    tmask = np.full((NF, 3, t_len), NEG, np.float32)
    for fi, fs in enumerate(FS):
        kf = min(Ldyn - fs + 1, t_len - fs + 1)
        if kf > 0:
            tmask[:, fi, :kf] = 0.0

    woutT = np.ascontiguousarray(
        np.asarray(inputs["Wout"], np.float32)[0].reshape(3, NF).T)
    bout = np.asarray(inputs["bout"], np.float32).reshape(1, 1)

    vt_full = (np.arange(t_len)[None, :] < new_lens[:, None]).astype(np.float32)

    in_maps = []
    for c in range(NCORES):
        rows = slice(c * BPC, (c + 1) * BPC)
        in_maps.append({
            "nembT": _pack_embT(new_emb[rows, :t_len], t_len).astype(np.float16),
            "wih0T": wih0T.astype(np.float16), "whh0T": whh0T.astype(np.float16),
            "bias0": bias0, "bhhn0": bhhn0,
            "wih1T": wih1T.astype(np.float16), "whh1T": whh1T.astype(np.float16),
            "bias1": bias1, "bhhn1": bhhn1,
            "vt": np.ascontiguousarray(vt_full[rows].reshape(1, BPC * t_len)),
            "cw": cw, "cb": cb, "tmask": tmask, "woutT": woutT, "bout": bout,
        })
    return in_maps, t_len


_NC_CACHE = {}


def _get_nc(which, t_len):
    key = (which, t_len)
    if key not in _NC_CACHE:
        _NC_CACHE[key] = build_kernel1(t_len) if which == 1 else build_kernel2(t_len)
    return _NC_CACHE[key]


TRACE = False  # set True (with an NTFF hook registered) to collect exec times
LAST_STATS = {}


def kernel(**inputs):
    from concourse import bass_utils

    gumbel = _gumbel_cpu()
    core_ids = list(range(NCORES))

    in_maps1, lens, maxlen, t_len1 = _host_pack_k1(inputs, gumbel)
    nc1 = _get_nc(1, t_len1)
    res1 = bass_utils.run_bass_kernel_spmd(nc1, in_maps1, core_ids, trace=TRACE)
    ks_full = np.zeros((B, T), np.float32)
    ks_full[:, :t_len1] = np.concatenate(
        [res1.results[c]["ks"].reshape(BPC, t_len1) for c in range(NCORES)], axis=0)

    new_emb, new_lens, Ldyn = _host_compact(inputs, ks_full, lens)
    in_maps2, t_len2 = _host_pack_k2(inputs, new_emb, new_lens, Ldyn)
    nc2 = _get_nc(2, t_len2)
    res2 = bass_utils.run_bass_kernel_spmd(nc2, in_maps2, core_ids, trace=TRACE)
    out = np.concatenate([res2.results[c]["out"][0] for c in range(NCORES)], axis=0)
    LAST_STATS["k1_ns"] = res1.exec_time_ns
    LAST_STATS["k2_ns"] = res2.exec_time_ns
    LAST_STATS["ks"] = ks_full
    LAST_STATS["new_lens"] = new_lens
    return out.astype(np.float32)


# revision 36
# speedup vs baseline: 1.0078x; 1.0078x over previous
"""Trainium2 Bass kernel for nn_CNN_RNN_88347477278730.

Pipeline (data-parallel over batch, 8 rows per core on 8 cores):
  kernel1 (fp32): input projection hoisted, then the select-policy GRUCell
      recurrence (t_len1 = maxlen-1 steps). Per step the gate PSUM is
      preloaded with gi_rz / bhh_n on the scalar engine and the h@Whh
      matmuls accumulate on top (start=False); the elementwise chain uses
      1-z = sigmoid(-x) so the critical path is 6 ops. All h_t are stored
      and the Gumbel decision bits are computed in bulk at the end.
  host: compaction (gather kept tokens to the front), new_lens, Ldyn.
  kernel2 (fp16 matmuls, t_len2 = Ldyn rounded up): proj of compacted
      embeddings, 2-layer GRU recurrences (h stored fp16), valid-masking,
      Kim-CNN convs as shifted matmuls, masked max-pool, final linear.

Matmul layouts are weights-stationary: lhsT = weight tiles [K=128, M=128],
moving operand = transposed activations [K, batch], so gate tensors land
partition-major where the elementwise engines are fast.
"""

import os
import subprocess
import sys
import tempfile

import numpy as np

# ---------------------------------------------------------------- constants
B, T, E, H, NF = 64, 512, 768, 256, 100
NCORES = 8
BPC = B // NCORES  # batch rows per core
KE = E // 128      # 6 K-tiles over the embedding dim
KH = H // 128      # 2 K-tiles over the hidden dim
GC = (3 * H) // 128  # 6 gate chunks (r: 0-1, z: 2-3, n: 4-5)
NEG = -1.0e30

FS = (3, 4, 5)


# ------------------------------------------------------------- tile patch
def _apply_tile_patch():
    """This walrus build rejects >2 sem waits on one SP control instruction;
    split the TileContext tail drain into several drains of <=2 waits."""
    import concourse.tile as tile
    from concourse.vector_clock import ScopedClock, VectorClock

    if getattr(tile.TileContext, "_drain_split_patched", False):
        return

    def _patched(self, tick_clock, wait_clock):
        gc = tick_clock.global_clock
        n = len(gc)
        for start in range(0, n, 1):
            vec = [0] * n
            any_set = False
            for p in range(start, min(start + 1, n)):
                vec[p] = gc[p]
                any_set = any_set or vec[p] > 0
            if not any_set:
                continue
            d = self.nc.sync.drain()
            wait_clock.add_sem_waits(d.ins, ScopedClock({None: VectorClock(vec)}))
        self.nc.all_engine_barrier()
        assert self.sems is not None
        popped = self.nc._tile_sem_poison_stack.pop()
        assert popped is self._sem_poison
        self.nc.clear_and_free_semaphores(list(self.sems.allocated().values()))
        self.nc.all_engine_barrier()

    tile.TileContext._drain_and_barrier = _patched
    tile.TileContext._drain_split_patched = True


# ------------------------------------------------------------- gumbel (CPU)
def _gumbel_cpu():
    """jax.random.gumbel(key(42), (T-1, B, 2), f32) — computed in a CPU-jax
    subprocess so the accelerator backend is never involved (it must be
    bit-identical to the reference's CPU computation)."""
    path = os.path.join(tempfile.mkdtemp(), "gumbel.npy")
    code = (
        "import numpy as np, jax, jax.numpy as jnp\n"
        f"g = jax.random.gumbel(jax.random.key(42), ({T - 1}, {B}, 2), jnp.float32)\n"
        f"np.save({path!r}, np.asarray(g))\n"
    )
    env = dict(os.environ)
    env["TRN_TERMINAL_POOL_IPS"] = ""
    env["JAX_PLATFORMS"] = "cpu"
    extra = [p for p in sys.path if p and os.path.isdir(p)]
    env["PYTHONPATH"] = os.pathsep.join(extra)
    subprocess.run([sys.executable, "-c", code], env=env, check=True, capture_output=True)
    return np.load(path)


# ------------------------------------------------------------- host packing
def _pack_T(a2d):
    """[rows(=128*k), cols] -> [128, k, cols] weight-tile layout."""
    rows, cols = a2d.shape
    k = rows // 128
    return np.ascontiguousarray(a2d.reshape(k, 128, cols).transpose(1, 0, 2)).astype(np.float32)


def _pack_bias(b1d):
    """[128*k] -> [128, k]"""
    k = b1d.shape[0] // 128
    return np.ascontiguousarray(b1d.reshape(k, 128).T).astype(np.float32)


def _pack_embT(emb_rows, t_len):
    """[bpc, t_len, E] -> [KE, 128, bpc*t_len] (e-major tiles, free (b, t))."""
    bpc = emb_rows.shape[0]
    x = emb_rows.transpose(2, 0, 1).reshape(KE, 128, bpc * t_len)
    return np.ascontiguousarray(x)


def _pack_gru_weights(Wih, Whh, bih, bhh):
    """Returns (wihT, whhT, bias_proj, bhhn_rep).

    bias_proj folds bih+bhh for the r,z chunks (added once at projection
    time); n chunks get bih only, with bhh_n preloaded into PSUM per step
    (it must be added to h@Whh_n *before* the r* multiply)."""
    wihT = _pack_T(np.ascontiguousarray(Wih.T))  # [128, KE or KH, 3H]
    whhT = _pack_T(np.ascontiguousarray(Whh.T))  # [128, KH, 3H]
    bias = np.empty(3 * H, np.float32)
    bias[: 2 * H] = bih[: 2 * H] + bhh[: 2 * H]
    bias[2 * H :] = bih[2 * H :]
    bias_proj = _pack_bias(bias)  # [128, GC]
    bhhn = _pack_bias(bhh[2 * H :])  # [128, KH]
    bhhn_rep = np.ascontiguousarray(
        np.broadcast_to(bhhn[:, :, None], (128, KH, BPC))
    ).astype(np.float32)
    return wihT, whhT, bias_proj, bhhn_rep


# ------------------------------------------------------------- bass builders
def _mk_nc():
    import concourse.bass as bass

    return bass.Bass("TRN2", target_bir_lowering=False, debug=False, num_devices=1)


def _split_excess_waits(nc, max_waits=1):
    """This walrus build can only encode ~2 sem waits per instruction
    (setupSyncWait 'Too many sync wait commands'). Hoist excess waits onto
    same-engine NoOps inserted just before the over-subscribed instruction;
    engine queues execute in order, so the wait semantics are identical."""
    from concourse import mybir

    nid = [0]
    for f in nc.m.functions:
        for bb in f.blocks:
            out = []
            changed = False
            for inst in bb.instructions:
                si = inst.sync_info
                lim = max_waits
                if si is not None and si.on_wait and len(si.on_wait) > lim:
                    waits = list(si.on_wait)
                    extra, keep = waits[:-lim], waits[-lim:]
                    for j in range(0, len(extra), max_waits):
                        nop = mybir.InstNoOp(
                            name=f"I-waitnop-{nid[0]}", ins=[], outs=[])
                        nid[0] += 1
                        nop.engine = inst.engine
                        nop.sync_info = mybir.SyncInfo(
                            on_wait=extra[j : j + max_waits], on_update=[])
                        nc.register_instruction(nop, overwrite=True)
                        out.append(nop)
                    inst.sync_info = mybir.SyncInfo(
                        on_wait=keep, on_update=list(si.on_update or []))
                    changed = True
                out.append(inst)
            if changed:
                bb.instructions = out
    return nc


def _proj_block_closures(nc, mybir, embT_d, wihcT, biasc, gi_rz, gin, lo, hi,
                         t_len, dma_pool, proj_ps, dt_in=None):
    """Closures that emit the input projection for t in [lo, hi).
    Returned as a list so the caller can pace them into the recurrence's
    PE idle gaps (each closure is one DMA / one matmul / one activation)."""
    f32 = mybir.dt.float32
    act = mybir.ActivationFunctionType
    if dt_in is None:
        dt_in = f32
    w = hi - lo
    cls = []
    for b in range(BPC):
        srch = []

        def _dma(srch=srch, b=b, lo=lo, w=w):
            src = dma_pool.tile([128, KE, w], dt_in, tag="projsrc", name="psrc")
            for k in range(KE):
                nc.sync.dma_start(
                    src[:, k, :],
                    embT_d[k, :, b * t_len + lo : b * t_len + lo + w])
            srch.append(src)

        cls.append(_dma)
        for c in range(GC):
            psh = []
            for k in range(KE):
                def _mm(srch=srch, psh=psh, c=c, k=k, w=w):
                    if k == 0:
                        psh.append(proj_ps.tile([128, w], f32, tag="projps", name="pps"))
                    nc.tensor.matmul(
                        psh[0][:], wihcT[:, k, c * 128 : (c + 1) * 128],
                        srch[0][:, k, :], start=(k == 0), stop=(k == KE - 1))
                cls.append(_mm)

            def _act(psh=psh, c=c, b=b, lo=lo, hi=hi):
                dst = gi_rz[:, lo:hi, c, b] if c < 4 else gin[:, lo:hi, c - 4, b]
                nc.scalar.activation(dst, psh[0][:], act.Identity,
                                     bias=biasc[:, c : c + 1])

            cls.append(_act)
    return cls


def _pace(sched, work, start, end):
    """Spread `work` closures evenly over steps [start, end]."""
    span = max(1, end - start)
    n = len(work)
    for j, cl in enumerate(work):
        sched.setdefault(start + (j * span) // n, []).append(cl)


def _emit_proj(nc, mybir, src_dram, wT_sb, bias_sb, gi_rz, gin, kin, t_len,
               dma_pool, ps_pool, dt_in):
    """gi[c*128+p, t, b] = sum_e W[e, c*128+p] * src[e, b, t] + bias.

    src_dram: DRAM [kin, 128, BPC*t_len]; wT_sb: [128, kin, 3H];
    gi_rz: [128, t_len, 4, BPC] (chunks 0-3); gin: [128, t_len, 2, BPC]."""
    f32 = mybir.dt.float32
    act = mybir.ActivationFunctionType
    for b in range(BPC):
        src_sb = dma_pool.tile([128, kin, t_len], dt_in, tag="projsrc")
        for k in range(kin):
            nc.sync.dma_start(
                src_sb[:, k, :], src_dram[k, :, b * t_len : (b + 1) * t_len]
            )
        for c in range(GC):
            ps = ps_pool.tile([128, t_len], f32, tag="projps")
            for k in range(kin):
                nc.tensor.matmul(
                    ps[:],
                    wT_sb[:, k, c * 128 : (c + 1) * 128],
                    src_sb[:, k, :],
                    start=(k == 0),
                    stop=(k == kin - 1),
                )
            dst = gi_rz[:, :, c, b] if c < 4 else gin[:, :, c - 4, b]
            nc.scalar.activation(dst, ps[:], act.Identity, bias=bias_sb[:, c : c + 1])


def _emit_gru_step(nc, mybir, t, h_prev, gi_rz, gin, whh_sb, bhhn_sb,
                   step_ps, sb_pool, h_out, fast_tail=False):
    """One GRU step. h_prev: AP [128, KH, BPC] of h_{t-1} (None for h=0).
    Writes h_t to h_out ([128, KH, BPC] view, any dtype).

    step_ps = (rz_pool, n_pool): the rz and n gate accumulations live in
    separate PSUM tiles so the rz add + sigmoid can start while the PE is
    still on the n-chunk matmuls. PSUM is written by the PE only — non-PE
    PSUM preloads raced with PE accumulation (nondeterministic results).

    fast_tail=False: baseline-exact numerics — gi added on DVE then sigmoid
    in place, and h' = n + z*(h-n). Required for the decision-exact select
    kernel. hn = psum + bhh_n is one DVE add (bit-identical to the Act
    bias-add it replaces, both are fp32 IEEE adds).
    fast_tail=True: 1-z = sigmoid(-zraw), h' = (1-z)*n + z*h with z*h off
    the critical path (1e-7-class deviation from the reference op order)."""
    f32 = mybir.dt.float32
    act = mybir.ActivationFunctionType
    alu = mybir.AluOpType

    rz_pool, n_pool = step_ps
    rz_ps = rz_pool.tile([128, 4, BPC], f32, tag="rzps")
    n_ps = n_pool.tile([128, 2, BPC], f32, tag="nps")
    if h_prev is not None:
        # chunk-major only: each column's accumulation group must be
        # contiguous — interleaving open groups in one PSUM bank corrupts
        # the accumulation (start resets bank-level group state)
        order = [(c, k) for c in range(GC) for k in range(KH)]
        for c, k in order:
            ps = rz_ps[:, c, :] if c < 4 else n_ps[:, c - 4, :]
            nc.tensor.matmul(
                ps,
                whh_sb[:, k, c * 128 : (c + 1) * 128],
                h_prev[:, k, :],
                start=(k == 0),
                stop=(k == KH - 1),
            )

    rz = sb_pool.tile([128, 4, BPC], f32, tag="rz")
    hn_src = bhhn_sb[:, :, :]
    if fast_tail:
        omz = sb_pool.tile([128, 2, BPC], f32, tag="omz")
        if h_prev is not None:
            raw = sb_pool.tile([128, 4, BPC], f32, tag="raw")
            nc.vector.tensor_tensor(raw[:], rz_ps[:], gi_rz[:, t, :, :], alu.add)
            nc.scalar.activation(rz[:], raw[:], act.Sigmoid)
            hn = sb_pool.tile([128, 2, BPC], f32, tag="hn")
            nc.vector.tensor_tensor(hn[:], n_ps[:], bhhn_sb, alu.add)
            hn_src = hn[:]
            nc.scalar.activation(omz[:], raw[:, 2:4, :], act.Sigmoid, scale=-1.0)
        else:
            nc.scalar.activation(rz[:], gi_rz[:, t, :, :], act.Sigmoid)
            nc.scalar.activation(omz[:], gi_rz[:, t, 2:4, :], act.Sigmoid, scale=-1.0)
        t1 = sb_pool.tile([128, 2, BPC], f32, tag="t1")
        nc.vector.tensor_tensor(t1[:], rz[:, 0:2, :], hn_src, alu.mult)
        t2 = sb_pool.tile([128, 2, BPC], f32, tag="t2")
        nc.vector.tensor_tensor(t2[:], t1[:], gin[:, t, :, :], alu.add)
        if h_prev is not None:
            # on gpsimd: off the serial path, keeps DVE free for the chain
            zh = sb_pool.tile([128, 2, BPC], f32, tag="zh")
            nc.gpsimd.tensor_tensor(zh[:], rz[:, 2:4, :], h_prev, alu.mult)
        nn_ = sb_pool.tile([128, 2, BPC], f32, tag="nn")
        nc.scalar.activation(nn_[:], t2[:], act.Tanh)
        if h_prev is None:
            nc.vector.tensor_tensor(h_out, omz[:], nn_[:], alu.mult)
        else:
            f1 = sb_pool.tile([128, 2, BPC], f32, tag="f1")
            nc.vector.tensor_tensor(f1[:], omz[:], nn_[:], alu.mult)
            nc.vector.tensor_tensor(h_out, f1[:], zh[:], alu.add)
        return

    # exact path (select kernel), k-split: the n-gate chain runs per
    # 128-dim half so the k0 half of h' lands ~1us before the k1 half and
    # the next step's k0 matmul round starts under the k1 tail. Per-element
    # arithmetic is identical to the unsplit form.
    if h_prev is not None:
        nc.vector.tensor_tensor(rz[:], rz_ps[:], gi_rz[:, t, :, :], alu.add)
        nc.scalar.activation(rz[:], rz[:], act.Sigmoid)
    else:
        nc.scalar.activation(rz[:], gi_rz[:, t, :, :], act.Sigmoid)
    t1 = sb_pool.tile([128, 2, BPC], f32, tag="t1")
    t2 = sb_pool.tile([128, 2, BPC], f32, tag="t2")
    nn_ = sb_pool.tile([128, 2, BPC], f32, tag="nn")
    hn = sb_pool.tile([128, 2, BPC], f32, tag="hn")
    for k in range(KH):
        if h_prev is not None:
            nc.vector.tensor_tensor(hn[:, k, :], n_ps[:, k, :],
                                    bhhn_sb[:, k, :], alu.add)
            hsrc = hn[:, k, :]
        else:
            hsrc = bhhn_sb[:, k, :]
        nc.vector.tensor_tensor(t1[:, k, :], rz[:, k, :], hsrc, alu.mult)
        nc.vector.tensor_tensor(t2[:, k, :], t1[:, k, :], gin[:, t, k, :], alu.add)
        nc.scalar.activation(nn_[:, k, :], t2[:, k, :], act.Tanh)
    d = sb_pool.tile([128, 2, BPC], f32, tag="dd")
    for k in range(KH):
        if h_prev is None:
            nc.vector.tensor_scalar(d[:, k, :], nn_[:, k, :], -1.0, None, alu.mult)
        else:
            nc.vector.tensor_tensor(d[:, k, :], h_prev[:, k, :], nn_[:, k, :],
                                    alu.subtract)
        nc.vector.tensor_tensor(d[:, k, :], rz[:, 2 + k, :], d[:, k, :], alu.mult)
        nc.vector.tensor_tensor(h_out[:, k, :], nn_[:, k, :], d[:, k, :], alu.add)


def build_kernel1(t_len):
    """Select-policy kernel: proj + recurrence + bulk decisions. fp32."""
    import concourse.tile as tile
    from concourse import mybir

    _apply_tile_patch()
    nc = _mk_nc()
    f32 = mybir.dt.float32
    alu = mybir.AluOpType

    embT_d = nc.dram_tensor("embT", [KE, 128, BPC * t_len], f32, kind="ExternalInput").ap()
    wihcT_d = nc.dram_tensor("wihcT", [128, KE, 3 * H], f32, kind="ExternalInput").ap()
    whhcT_d = nc.dram_tensor("whhcT", [128, KH, 3 * H], f32, kind="ExternalInput").ap()
    wdiffT_d = nc.dram_tensor("wdiffT", [128, KH, 1], f32, kind="ExternalInput").ap()
    biasc_d = nc.dram_tensor("biasc", [128, GC], f32, kind="ExternalInput").ap()
    bhhnc_d = nc.dram_tensor("bhhnc", [128, KH, BPC], f32, kind="ExternalInput").ap()
    ncdiff_d = nc.dram_tensor("ncdiff", [1, BPC * t_len], f32, kind="ExternalInput").ap()
    ks_d = nc.dram_tensor("ks", [1, BPC * t_len], f32, kind="ExternalOutput").ap()

    with tile.TileContext(nc) as tc:
        from contextlib import ExitStack

        with ExitStack() as ctx:
            wpool = ctx.enter_context(tc.tile_pool(name="weights", bufs=1))
            gipool = ctx.enter_context(tc.tile_pool(name="gi", bufs=1))
            dma_pool = ctx.enter_context(tc.tile_pool(name="dma", bufs=2))
            proj_ps = ctx.enter_context(tc.tile_pool(name="projps", bufs=2, space="PSUM"))
            rz_pool = ctx.enter_context(tc.tile_pool(name="rzps", bufs=2, space="PSUM"))
            n_pool = ctx.enter_context(tc.tile_pool(name="nps", bufs=2, space="PSUM"))
            step_ps = (rz_pool, n_pool)
            lg_ps = ctx.enter_context(tc.tile_pool(name="lgps", bufs=2, space="PSUM"))
            sb_pool = ctx.enter_context(tc.tile_pool(name="gates", bufs=2))
            misc = ctx.enter_context(tc.tile_pool(name="misc", bufs=1))

            def _load(pool, dram, shape, tag, dt=f32):
                t_ = pool.tile(shape, dt, tag=tag)
                nc.sync.dma_start(t_[:], dram[:])
                return t_

            wihcT = _load(wpool, wihcT_d, [128, KE, 3 * H], "wihcT")
            whhcT = _load(wpool, whhcT_d, [128, KH, 3 * H], "whhcT")
            wdiffT = _load(wpool, wdiffT_d, [128, KH, 1], "wdiffT")
            biasc = _load(wpool, biasc_d, [128, GC], "biasc")
            bhhnc = _load(wpool, bhhnc_d, [128, KH, BPC], "bhhnc")
            ncdiff = _load(misc, ncdiff_d, [1, BPC * t_len], "ncdiff")

            gi_rz = gipool.tile([128, t_len, 4, BPC], f32, tag="girz")
            gin = gipool.tile([128, t_len, 2, BPC], f32, tag="gin")
            h_all = gipool.tile([128, KH, BPC, t_len], f32, tag="hall")

            # t-blocked proj: block 0 upfront, later blocks paced into the
            # recurrence's PE idle gaps (block i must land before step bnds[i])
            bnds = list(range(0, t_len, 128))
            if bnds[-1] != t_len:
                bnds.append(t_len)
            blocks = [(bnds[i], bnds[i + 1]) for i in range(len(bnds) - 1)]
            for cl in _proj_block_closures(nc, mybir, embT_d, wihcT, biasc,
                                           gi_rz, gin, blocks[0][0], blocks[0][1],
                                           t_len, dma_pool, proj_ps):
                cl()
            sched = {}
            for i in range(1, len(blocks)):
                lo, hi = blocks[i]
                work = _proj_block_closures(nc, mybir, embT_d, wihcT, biasc,
                                            gi_rz, gin, lo, hi, t_len,
                                            dma_pool, proj_ps)
                _pace(sched, work, blocks[i - 1][0] + 2, lo - 4)

            h_prev = None
            for t in range(t_len):
                h_out = h_all[:, :, :, t]
                _emit_gru_step(nc, mybir, t, h_prev, gi_rz, gin, whhcT, bhhnc,
                               step_ps, sb_pool, h_out)
                for cl in sched.pop(t, []):
                    cl()
                h_prev = h_out
            for rest in sorted(sched):
                for cl in sched.pop(rest):
                    cl()

            # bulk decision readout: ks[b, t] = (h_t . wdiff > ncdiff[b, t])
            ks_sb = misc.tile([1, BPC * t_len], f32, tag="kssb")
            for b in range(BPC):
                lg = lg_ps.tile([1, t_len], f32, tag="lg")
                for k in range(KH):
                    nc.tensor.matmul(
                        lg[:],
                        wdiffT[:, k, :],
                        h_all[:, k, b, :],
                        start=(k == 0),
                        stop=(k == KH - 1),
                    )
                nc.vector.tensor_tensor(
                    ks_sb[0:1, b * t_len : (b + 1) * t_len], lg[:],
                    ncdiff[0:1, b * t_len : (b + 1) * t_len], alu.is_gt
                )
            nc.sync.dma_start(ks_d[:], ks_sb[:])

    return _split_excess_waits(nc)


def build_kernel2(t_len):
    """GRU0/GRU1 + convs + pooling + final linear. fp16 matmuls."""
    import concourse.tile as tile
    from concourse import mybir

    _apply_tile_patch()
    nc = _mk_nc()
    f32 = mybir.dt.float32
    f16 = mybir.dt.float16
    act = mybir.ActivationFunctionType
    alu = mybir.AluOpType

    nembT_d = nc.dram_tensor("nembT", [KE, 128, BPC * t_len], f16, kind="ExternalInput").ap()
    wih0T_d = nc.dram_tensor("wih0T", [128, KE, 3 * H], f16, kind="ExternalInput").ap()
    whh0T_d = nc.dram_tensor("whh0T", [128, KH, 3 * H], f16, kind="ExternalInput").ap()
    bias0_d = nc.dram_tensor("bias0", [128, GC], f32, kind="ExternalInput").ap()
    bhhn0_d = nc.dram_tensor("bhhn0", [128, KH, BPC], f32, kind="ExternalInput").ap()
    wih1T_d = nc.dram_tensor("wih1T", [128, KH, 3 * H], f16, kind="ExternalInput").ap()
    whh1T_d = nc.dram_tensor("whh1T", [128, KH, 3 * H], f16, kind="ExternalInput").ap()
    bias1_d = nc.dram_tensor("bias1", [128, GC], f32, kind="ExternalInput").ap()
    bhhn1_d = nc.dram_tensor("bhhn1", [128, KH, BPC], f32, kind="ExternalInput").ap()
    vt_d = nc.dram_tensor("vt", [1, BPC * t_len], f32, kind="ExternalInput").ap()
    cw_d = nc.dram_tensor("cw", [128, 12, KH, NF], f16, kind="ExternalInput").ap()
    cb_d = nc.dram_tensor("cb", [NF, 3], f32, kind="ExternalInput").ap()
    tmask_d = nc.dram_tensor("tmask", [NF, 3, t_len], f32, kind="ExternalInput").ap()
    woutT_d = nc.dram_tensor("woutT", [NF, 3], f32, kind="ExternalInput").ap()
    bout_d = nc.dram_tensor("bout", [1, 1], f32, kind="ExternalInput").ap()
    out_d = nc.dram_tensor("out", [1, BPC], f32, kind="ExternalOutput").ap()

    with tile.TileContext(nc) as tc:
        from contextlib import ExitStack

        with ExitStack() as ctx:
            wpool = ctx.enter_context(tc.tile_pool(name="weights", bufs=1))
            gipool = ctx.enter_context(tc.tile_pool(name="gi", bufs=1))
            opool = ctx.enter_context(tc.tile_pool(name="obuf", bufs=1))
            dma_pool = ctx.enter_context(tc.tile_pool(name="dma", bufs=2))
            proj_ps = ctx.enter_context(tc.tile_pool(name="projps", bufs=2, space="PSUM"))
            rz_pool = ctx.enter_context(tc.tile_pool(name="rzps", bufs=2, space="PSUM"))
            n_pool = ctx.enter_context(tc.tile_pool(name="nps", bufs=2, space="PSUM"))
            step_ps = (rz_pool, n_pool)
            fin_ps = ctx.enter_context(tc.tile_pool(name="finps", bufs=1, space="PSUM"))
            sb_pool = ctx.enter_context(tc.tile_pool(name="gates", bufs=3))
            misc = ctx.enter_context(tc.tile_pool(name="misc", bufs=1))

            def _load(pool, dram, shape, tag, dt=f32):
                t_ = pool.tile(shape, dt, tag=tag)
                nc.sync.dma_start(t_[:], dram[:])
                return t_

            wih0T = _load(wpool, wih0T_d, [128, KE, 3 * H], "bigw", f16)
            whh0T = _load(wpool, whh0T_d, [128, KH, 3 * H], "whh0", f16)
            bias0 = _load(wpool, bias0_d, [128, GC], "bias0")
            bhhn0 = _load(wpool, bhhn0_d, [128, KH, BPC], "bhhn0")
            wih1T = _load(wpool, wih1T_d, [128, KH, 3 * H], "wih1", f16)
            whh1T = _load(wpool, whh1T_d, [128, KH, 3 * H], "whh1", f16)
            bias1 = _load(wpool, bias1_d, [128, GC], "bias1")
            bhhn1 = _load(wpool, bhhn1_d, [128, KH, BPC], "bhhn1")
            cb = _load(misc, cb_d, [NF, 3], "cb")
            tmask = _load(misc, tmask_d, [NF, 3, t_len], "tmask")
            woutT = _load(misc, woutT_d, [NF, 3], "woutT")
            bout = _load(misc, bout_d, [1, 1], "bout")
            vt = _load(misc, vt_d, [1, BPC * t_len], "vt")

            # ---- layer 0 + pipelined layer 1 (lagged by LAG steps) ----
            gi_rz0 = gipool.tile([128, t_len, 4, BPC], f32, tag="girz0")
            gin0 = gipool.tile([128, t_len, 2, BPC], f32, tag="gin0")
            _emit_proj(nc, mybir, nembT_d, wih0T, bias0, gi_rz0, gin0, KE, t_len,
                       dma_pool, proj_ps, f16)
            o1 = opool.tile([128, KH, BPC, t_len], f16, tag="o1")
            o2 = opool.tile([128, KH, BPC, t_len], f16, tag="o2")
            gi_rz1 = gipool.tile([128, t_len, 4, BPC], f32, tag="girz1")
            gin1 = gipool.tile([128, t_len, 2, BPC], f32, tag="gin1")

            PB, LAG = 64, 96

            def proj1_block_closures(lo, hi):
                cls = []
                for b in range(BPC):
                    for c in range(GC):
                        psh = []
                        for k in range(KH):
                            def _mm(psh=psh, c=c, b=b, k=k, lo=lo, hi=hi):
                                if k == 0:
                                    psh.append(proj_ps.tile(
                                        [128, hi - lo], f32, tag="projps",
                                        name="pps1"))
                                nc.tensor.matmul(
                                    psh[0][:], wih1T[:, k, c * 128 : (c + 1) * 128],
                                    o1[:, k, b, lo:hi],
                                    start=(k == 0), stop=(k == KH - 1))
                            cls.append(_mm)

                        def _act(psh=psh, c=c, b=b, lo=lo, hi=hi):
                            dst = (gi_rz1[:, lo:hi, c, b] if c < 4
                                   else gin1[:, lo:hi, c - 4, b])
                            nc.scalar.activation(dst, psh[0][:], act.Identity,
                                                 bias=bias1[:, c : c + 1])
                        cls.append(_act)
                return cls

            sched = {}
            lo = 0
            while lo < t_len:
                hi = min(lo + PB, t_len)
                # o1[:, lo:hi] complete after layer-0 step hi-1; gi1[lo:hi]
                # needed by layer-1 step lo, which runs at merged step lo+LAG
                _pace(sched, proj1_block_closures(lo, hi), hi, lo + LAG - 2)
                lo = hi

            h0_prev = None
            h1_prev = None
            for tt in range(t_len + LAG):
                if tt < t_len:
                    h_out = o1[:, :, :, tt]
                    _emit_gru_step(nc, mybir, tt, h0_prev, gi_rz0, gin0, whh0T,
                                   bhhn0, step_ps, sb_pool, h_out, fast_tail=True)
                    h0_prev = h_out
                for cl in sched.pop(tt, []):
                    cl()
                if tt >= LAG:
                    t1_ = tt - LAG
                    h_out = o2[:, :, :, t1_]
                    _emit_gru_step(nc, mybir, t1_, h1_prev, gi_rz1, gin1, whh1T,
                                   bhhn1, step_ps, sb_pool, h_out, fast_tail=True)
                    h1_prev = h_out

            # ---- zero o2 past new_lens: o2 *= vt ----
            ones_sb = misc.tile([1, 128], f32, tag="ones")
            nc.vector.memset(ones_sb[:], 1.0)
            for b in range(BPC):
                vtb = proj_ps.tile([128, t_len], f32, tag="projps")
                nc.tensor.matmul(
                    vtb[:], ones_sb[:], vt[0:1, b * t_len : (b + 1) * t_len],
                    start=True, stop=True,
                )
                for k in range(KH):
                    nc.vector.tensor_tensor(
                        o2[:, k, b, :], o2[:, k, b, :], vtb[:], alu.mult
                    )

            # ---- convs + relu + tmask + max-pool ----
            cw = _load(wpool, cw_d, [128, 12, KH, NF], "bigw", f16)
            pooled = misc.tile([NF, 3, BPC], f32, tag="pooled")
            for b in range(BPC):
                for fi, fs in enumerate(FS):
                    nw = t_len - fs + 1
                    ps = proj_ps.tile([NF, t_len], f32, tag="projps")
                    m0 = sum(FS[:fi])
                    first = True
                    for dt_ in range(fs):
                        for k in range(KH):
                            nc.tensor.matmul(
                                ps[:, :nw],
                                cw[:, m0 + dt_, k, :],
                                o2[:, k, b, dt_ : dt_ + nw],
                                start=first,
                                stop=(dt_ == fs - 1 and k == KH - 1),
                            )
                            first = False
                    crelu = sb_pool.tile([NF, t_len], f32, tag="crelu")
                    nc.scalar.activation(
                        crelu[:, :nw], ps[:, :nw], act.Relu, bias=cb[:, fi : fi + 1]
                    )
                    nc.vector.tensor_tensor(
                        crelu[:, :nw], crelu[:, :nw], tmask[:, fi, :nw], alu.add
                    )
                    nc.vector.tensor_reduce(
                        pooled[:, fi, b : b + 1], crelu[:, :nw],
                        mybir.AxisListType.X, alu.max
                    )

            # ---- final linear ----
            fps = fin_ps.tile([1, BPC], f32)
            for fi in range(3):
                nc.tensor.matmul(
                    fps[:],
                    woutT[:, fi : fi + 1],
                    pooled[:, fi, :],
                    start=(fi == 0),
                    stop=(fi == 2),
                )
            out_sb = misc.tile([1, BPC], f32, tag="outsb")
            nc.scalar.activation(out_sb[:], fps[:], act.Identity, bias=bout[0:1, 0:1])
            nc.sync.dma_start(out_d[:], out_sb[:])

    return _split_excess_waits(nc)


# ------------------------------------------------------------- host orchestration
def _host_pack_k1(inputs, gumbel):
    emb = np.asarray(inputs["embedded"], np.float32)
    mask = np.asarray(inputs["mask"])
    lens = mask.sum(1)
    maxlen = int(lens.max())
    t_len = maxlen - 1  # h_t needed only for t <= maxlen-2; +1 for t=0 row

    wihcT, whhcT, biasc, bhhnc = _pack_gru_weights(
        inputs["Wih_c"], inputs["Whh_c"], inputs["bih_c"], inputs["bhh_c"])
    wdiff = (inputs["Wsel"][1] - inputs["Wsel"][0]).astype(np.float32)
    wdiffT = np.ascontiguousarray(
        wdiff.reshape(KH, 128).T[:, :, None]).astype(np.float32)
    bdiff = float(inputs["bsel"][1] - inputs["bsel"][0])

    # ncdiff[t, b]: k_t = (h_t . wdiff > ncdiff); t=0 is not a decision
    ncdiff = np.full((t_len, B), 1.0e30, np.float32)
    for t in range(1, t_len):
        ncdiff[t] = -(bdiff + gumbel[t - 1, :, 1] - gumbel[t - 1, :, 0])

    in_maps = []
    for c in range(NCORES):
        rows = slice(c * BPC, (c + 1) * BPC)
        in_maps.append({
            "embT": _pack_embT(emb[rows, :t_len], t_len).astype(np.float32),
            "wihcT": wihcT,
            "whhcT": whhcT,
            "wdiffT": wdiffT,
            "biasc": biasc,
            "bhhnc": bhhnc,
            "ncdiff": np.ascontiguousarray(ncdiff[:, rows].T.reshape(1, BPC * t_len)),
        })
    return in_maps, lens, maxlen, t_len


def _host_compact(inputs, ks_full, lens):
    """ks_full: [B, T] decision bits (col 0 ignored; selected[:,0]=1)."""
    emb = np.asarray(inputs["embedded"], np.float32)
    selected = np.zeros((B, T), np.int64)
    selected[:, 0] = 1
    selected[:, 1:] = ks_full[:, 1:]
    pos = np.arange(T)
    sel_valid = np.where(pos[None, :] < (lens - 1)[:, None], selected, 0)
    new_mask = np.where(pos[None, :] == (lens - 1)[:, None], 1, sel_valid)
    new_lens = new_mask.sum(1)
    Ldyn = max(int(new_lens.max()), 7)

    new_emb = np.zeros((B, T, E), np.float32)
    for b in range(B):
        idx = np.nonzero(new_mask[b])[0]
        new_emb[b, : len(idx)] = emb[b, idx]
    return new_emb, new_lens, Ldyn


def _host_pack_k2(inputs, new_emb, new_lens, Ldyn):
    t_len = min(max(16 * ((Ldyn + 15) // 16), 32), T)

    wih0T, whh0T, bias0, bhhn0 = _pack_gru_weights(
        inputs["Wih0"], inputs["Whh0"], inputs["bih0"], inputs["bhh0"])
    wih1T, whh1T, bias1, bhhn1 = _pack_gru_weights(
        inputs["Wih1"], inputs["Whh1"], inputs["bih1"], inputs["bhh1"])

    cw = np.zeros((128, 12, KH, NF), np.float16)
    cb = np.zeros((NF, 3), np.float32)
    m = 0
    for fi, fs in enumerate(FS):
        w = np.asarray(inputs[f"conv_w{fs}"], np.float32)  # [NF,1,fs,H]
        cb[:, fi] = np.asarray(inputs[f"conv_b{fs}"], np.float32)
        for dt_ in range(fs):
            wt = w[:, 0, dt_, :].T  # [H, NF]
            cw[:, m, :, :] = wt.reshape(KH, 128, NF).transpose(1, 0, 2).astype(np.float16)
            m += 1

    tmask = np.full((NF, 3, t_len), NEG, np.float32)
    for fi, fs in enumerate(FS):
        kf = min(Ldyn - fs + 1, t_len - fs + 1)
        if kf > 0:
            tmask[:, fi, :kf] = 0.0

    woutT = np.ascontiguousarray(
        np.asarray(inputs["Wout"], np.float32)[0].reshape(3, NF).T)
    bout = np.asarray(inputs["bout"], np.float32).reshape(1, 1)

    vt_full = (np.arange(t_len)[None, :] < new_lens[:, None]).astype(np.float32)

    in_maps = []
    for c in range(NCORES):
        rows = slice(c * BPC, (c + 1) * BPC)
        in_maps.append({
            "nembT": _pack_embT(new_emb[rows, :t_len], t_len).astype(np.float16),
            "wih0T": wih0T.astype(np.float16), "whh0T": whh0T.astype(np.float16),
            "bias0": bias0, "bhhn0": bhhn0,
            "wih1T": wih1T.astype(np.float16), "whh1T": whh1T.astype(np.float16),
            "bias1": bias1, "bhhn1": bhhn1,
            "vt": np.ascontiguousarray(vt_full[rows].reshape(1, BPC * t_len)),
            "cw": cw, "cb": cb, "tmask": tmask, "woutT": woutT, "bout": bout,
        })
    return in_maps, t_len


_NC_CACHE = {}


def _get_nc(which, t_len):
    key = (which, t_len)
    if key not in _NC_CACHE:
        _NC_CACHE[key] = build_kernel1(t_len) if which == 1 else build_kernel2(t_len)
    return _NC_CACHE[key]


TRACE = False  # set True (with an NTFF hook registered) to collect exec times
LAST_STATS = {}


def kernel(**inputs):
    from concourse import bass_utils

    gumbel = _gumbel_cpu()
    core_ids = list(range(NCORES))

    in_maps1, lens, maxlen, t_len1 = _host_pack_k1(inputs, gumbel)
    nc1 = _get_nc(1, t_len1)
    res1 = bass_utils.run_bass_kernel_spmd(nc1, in_maps1, core_ids, trace=TRACE)
    ks_full = np.zeros((B, T), np.float32)
    ks_full[:, :t_len1] = np.concatenate(
        [res1.results[c]["ks"].reshape(BPC, t_len1) for c in range(NCORES)], axis=0)

    new_emb, new_lens, Ldyn = _host_compact(inputs, ks_full, lens)
    in_maps2, t_len2 = _host_pack_k2(inputs, new_emb, new_lens, Ldyn)
    nc2 = _get_nc(2, t_len2)
    res2 = bass_utils.run_bass_kernel_spmd(nc2, in_maps2, core_ids, trace=TRACE)
    out = np.concatenate([res2.results[c]["out"][0] for c in range(NCORES)], axis=0)
    LAST_STATS["k1_ns"] = res1.exec_time_ns
    LAST_STATS["k2_ns"] = res2.exec_time_ns
    LAST_STATS["ks"] = ks_full
    LAST_STATS["new_lens"] = new_lens
    return out.astype(np.float32)
